# revision 24
# baseline (speedup 1.0000x reference)
"""Distance-correlation (DisCo) loss kernel for Trainium2, sharded over 8 NeuronCores.

Math: reference computes NxN pairwise |vi-vj| matrices (a, b), weighted row
means, double-centering, then scalar reductions.  Everything except the
genuinely 2-D term

    Q_ab[i] = sum_j w_j * |v1_i - v1_j| * |v2_i - v2_j|

has an exact O(N log N) closed form on the host (sorted prefix sums for
weighted |.| row sums, polynomial identities for squared terms).  The device
computes Q_ab only, with rows i sharded across the 8 cores (1024 rows/core).

Device layout (per core): i on partitions (8 blocks of 128), j on the free
dim (2 chunks of 4096).  Tiles a=|v1_i-v1_j| and b=|v2_i-v2_j| are built by
the Vector engine (tensor_scalar: abs_max(in0-s, 0), fp32 2x mode) and the
Scalar engine (activation Abs with per-partition bias) splitting the j
columns, then a fused tensor_tensor_reduce (mult + free-dim add) produces
the row sums, chained across j chunks.  The weighted fallback multiplies b
by a broadcast w tile first.
"""

import functools
import os

import numpy as np

N = 8192
CORES = 8
ROWS = N // CORES          # 1024 rows per core
NIB = ROWS // 128          # 8 partition blocks per core
BCH = 1024                 # broadcast DMA chunk

LAST_RESULT = None         # BassKernelResults of the most recent launch


@functools.lru_cache(maxsize=1)
def _disco_op():
    """Fused DVE op: out = |in0-s0| * |in1-s1|, accum_out = sum(out).

    Registered at runtime into concourse.dve_ops; the uop table ships in
    the NEFF, so no firmware support is needed.  Exactly fills the 8-stage
    v3 DVE pipeline (2 subs, 2 negates, 2 maxes, 1 mul, 1 accum-add).
    """
    from operator import add

    import concourse.dve_ops as D
    from concourse.dve_spec import Spec, Src0, Src1, C0, C1, Zero, maxx, lower
    from concourse.dve_uop import DveOpSpec

    d1 = Src0 - C0
    d2 = Src1 - C1
    body = maxx(d1, Zero - d1) * maxx(d2, Zero - d2)

    def ref(in0, in1, s0, s1, imm2):
        b = (
            np.abs(in0.astype(np.float32) - s0) * np.abs(in1.astype(np.float32) - s1)
        ).astype(np.float32)
        return b, b.reshape(b.shape[0], -1).sum(axis=-1, keepdims=True)

    spec = Spec(body=body, accum=add, accum_init=Zero, reference=ref)
    name = "DISCO_ABSPROD_REDUCE"
    row = max(D._SUB_OPCODE_FOR_NAME.values()) + 1
    D._SUB_OPCODE_FOR_NAME[name] = row
    sha3 = DveOpSpec(
        name=name, opcode=row, uops=lower(spec, ver="v3"), rd1_en=True
    ).sha("v3")
    op = D.DveOp(name, spec, subdim=False, uops_sha={"v3": sha3})
    D.OPS.append(op)
    D.CUSTOM_DVE_SPECS[name] = spec
    return op


@functools.lru_cache(maxsize=3)
def _build(mode: str):
    """mode: 'sym' (w==1, symmetric block-triangle), 'full' (w==1, full
    matrix), or 'weighted' (general w)."""
    if mode == "sym":
        return _build_sym()
    import concourse.bacc as bacc
    import concourse.bass as bass
    import concourse.tile as tile
    from concourse import mybir

    weighted = mode == "weighted"
    f32 = mybir.dt.float32
    nc = bacc.Bacc("TRN2", target_bir_lowering=False, debug=False)

    # j-chunk size and the VectorE share of build columns, chosen to balance
    # VectorE vs ScalarE busy time per chunk while fitting SBUF.
    JC = 2048
    JD = 0
    NJC = N // JC

    v1d = nc.dram_tensor("v1", [N], f32, kind="ExternalInput")
    v2d = nc.dram_tensor("v2", [N], f32, kind="ExternalInput")
    wd = nc.dram_tensor("w", [N], f32, kind="ExternalInput") if weighted else None
    # vipack columns: [vi1 | -vi1 | vi2 | -vi2], each NIB wide, partition-major.
    vipackd = nc.dram_tensor("vipack", [128, 4 * NIB], f32, kind="ExternalInput")
    if weighted:
        qabd = nc.dram_tensor("qab", [128, NIB], f32, kind="ExternalOutput")
    else:
        qabd = nc.dram_tensor("qab", [128, NIB, NJC], f32, kind="ExternalOutput")

    def bcast(ap1d):
        return bass.AP(
            tensor=ap1d.tensor, offset=ap1d.offset, ap=[[0, 128]] + list(ap1d.ap)
        )

    i32 = mybir.dt.int32
    sub = mybir.AluOpType.subtract
    band = mybir.AluOpType.bitwise_and
    mult = mybir.AluOpType.mult
    add = mybir.AluOpType.add

    with tile.TileContext(nc) as tc:
        with (
            tc.tile_pool(name="singles", bufs=1) as singles,
            tc.tile_pool(name="ab", bufs=2) as pab,
            tc.tile_pool(name="scrap", bufs=1) as pscrap,
        ):
            v1rep = singles.tile([128, N], f32)
            v2rep = singles.tile([128, N], f32)
            reps = [(v1rep, v1d), (v2rep, v2d)]
            wrep = None
            if weighted:
                wrep = singles.tile([128, N], f32)
                reps.append((wrep, wd))
            for c in range(N // BCH):
                for rep, src in reps:
                    sap = src.ap()
                    nc.sync.dma_start(
                        out=rep[:, c * BCH : (c + 1) * BCH],
                        in_=bcast(sap[c * BCH : (c + 1) * BCH]),
                    )

            vipack = singles.tile([128, 4 * NIB], f32)
            nc.sync.dma_start(out=vipack[:, :], in_=vipackd.ap())
            vi1 = vipack[:, 0 * NIB : 1 * NIB]
            nvi1 = vipack[:, 1 * NIB : 2 * NIB]
            vi2 = vipack[:, 2 * NIB : 3 * NIB]
            nvi2 = vipack[:, 3 * NIB : 4 * NIB]

            if not weighted:
                # fused path: one custom DVE op per (i-block, chunk) computes
                # |v1_j - v1_i| * |v2_j - v2_i| and its row sum directly from
                # the replicated source rows -- no build tiles at all.
                op = _disco_op()
                qacc2 = singles.tile([128, NIB, NJC], f32)
                for jc in range(NJC):
                    for ib in range(NIB):
                        j0 = jc * JC
                        scrap = pscrap.tile([128, JC], f32)
                        nc.vector._custom_dve(
                            op,
                            out=scrap[:, :],
                            in0=v1rep[:, j0 : j0 + JC],
                            in1=v2rep[:, j0 : j0 + JC],
                            s0=vi1[:, ib : ib + 1],
                            s1=vi2[:, ib : ib + 1],
                            accum_out=qacc2[:, ib, jc : jc + 1],
                        )
                nc.sync.dma_start(out=qabd.ap(), in_=qacc2[:, :, :])
            else:
                qacc = singles.tile([128, NIB], f32)
                mask = None
                if JD > 0:
                    # 0x7FFFFFFF sign-clear mask: |x| on VectorE is a fp32
                    # subtract followed by an int32 bitwise_and against this.
                    mask = singles.tile([128, JD], i32)
                    nc.vector.memset(mask, 0x7FFFFFFF)

                for ib in range(NIB):
                    for jc in range(NJC):
                        j0 = jc * JC
                        ab = pab.tile([128, 2, JC], f32, tag="ab")
                        a = ab[:, 0, :]
                        b = ab[:, 1, :]
                        for t, (rep, vis, nvis) in enumerate(
                            ((v1rep, vi1, nvi1), (v2rep, vi2, nvi2))
                        ):
                            if JD > 0:
                                nc.vector.tensor_scalar(
                                    ab[:, t, :JD],
                                    rep[:, j0 : j0 + JD],
                                    vis[:, ib : ib + 1],
                                    None,
                                    sub,
                                )
                            nc.scalar.activation(
                                out=ab[:, t, JD:],
                                in_=rep[:, j0 + JD : j0 + JC],
                                func=mybir.ActivationFunctionType.Abs,
                                bias=nvis[:, ib : ib + 1],
                                scale=1.0,
                            )
                        if JD > 0:
                            for t in range(2):
                                nc.vector.tensor_tensor(
                                    ab[:, t, :JD].bitcast(i32),
                                    ab[:, t, :JD].bitcast(i32),
                                    mask[:, :],
                                    band,
                                )
                        wb = pab.tile([128, JC], f32, tag="wb")
                        nc.vector.tensor_tensor(wb, b, wrep[:, j0 : j0 + JC], mult)
                        scrap = pscrap.tile([128, JC], f32)
                        nc.vector.tensor_tensor(scrap, a, wb, mult)
                        # in-place copy whose op1 performs the free-dim
                        # reduction, chained across j chunks via scalar2 init
                        nc.vector.tensor_scalar(
                            scrap,
                            scrap,
                            1.0,
                            (0.0 if jc == 0 else qacc[:, ib : ib + 1]),
                            mult,
                            add,
                            accum_out=qacc[:, ib : ib + 1],
                        )

                nc.sync.dma_start(out=qabd.ap(), in_=qacc[:, :])

    nc.compile()
    return nc


def _build_sym():
    """Symmetric fast path (w == 1).

    M_ij = |v1_i-v1_j|*|v2_i-v2_j| is symmetric, so only the block upper
    triangle is computed.  Rows are interleaved across cores (core c owns
    global rows r with r % 8 == c), so each core's i-block b covers the
    global 1024-row band b.  Per band: the diagonal 1024-wide j-band is
    computed in full (row sums only); bands jc > b are computed once, with
    the custom op's accumulator providing the row sums and a ones-vector
    TensorE matmul (f32r, full rate) providing the partition-dim column
    sums into PSUM, accumulated over b and DMA'd out per jc.  0.5625x the
    elementwise work of the full matrix.
    """
    import concourse.bacc as bacc
    import concourse.bass as bass
    import concourse.tile as tile
    from concourse import mybir

    f32 = mybir.dt.float32
    f32r = mybir.dt.float32r
    BD = 1024  # band width
    NB = N // BD  # 8 bands == NIB
    assert NB == NIB

    nc = bacc.Bacc("TRN2", target_bir_lowering=False, debug=False)
    v1d = nc.dram_tensor("v1", [N], f32, kind="ExternalInput")
    v2d = nc.dram_tensor("v2", [N], f32, kind="ExternalInput")
    vipackd = nc.dram_tensor("vipack", [128, 4 * NIB], f32, kind="ExternalInput")
    qrowd = nc.dram_tensor("qrow", [128, NIB, NB], f32, kind="ExternalOutput")
    qcold = nc.dram_tensor("qcol", [NB - 1, BD], f32, kind="ExternalOutput")

    def bcast(ap1d):
        return bass.AP(
            tensor=ap1d.tensor, offset=ap1d.offset, ap=[[0, 128]] + list(ap1d.ap)
        )

    op = _disco_op()
    with tile.TileContext(nc) as tc:
        with (
            tc.tile_pool(name="singles", bufs=1) as singles,
            tc.tile_pool(name="scrap", bufs=4) as pscrap,
            tc.tile_pool(name="psum", bufs=2, space="PSUM") as ppsum,
        ):
            v1rep = singles.tile([128, N], f32)
            v2rep = singles.tile([128, N], f32)
            for c in range(N // BCH):
                for rep, src in ((v1rep, v1d), (v2rep, v2d)):
                    sap = src.ap()
                    nc.sync.dma_start(
                        out=rep[:, c * BCH : (c + 1) * BCH],
                        in_=bcast(sap[c * BCH : (c + 1) * BCH]),
                    )
            vipack = singles.tile([128, 4 * NIB], f32)
            nc.sync.dma_start(out=vipack[:, :], in_=vipackd.ap())
            vi1 = vipack[:, 0 * NIB : 1 * NIB]
            vi2 = vipack[:, 2 * NIB : 3 * NIB]

            qacc = singles.tile([128, NIB, NB], f32)
            nc.vector.memset(qacc, 0.0)
            ones_f = singles.tile([128, 1], f32)
            nc.vector.memset(ones_f, 1.0)
            ones = singles.tile([128, 1], f32r)
            nc.vector.tensor_copy(ones[:, :], ones_f[:, :])

            for jc in range(NB):
                j0 = jc * BD
                # diagonal band: full row, row sums only
                scrap = pscrap.tile([128, BD], f32, tag="scrap")
                nc.vector._custom_dve(
                    op,
                    out=scrap[:, :],
                    in0=v1rep[:, j0 : j0 + BD],
                    in1=v2rep[:, j0 : j0 + BD],
                    s0=vi1[:, jc : jc + 1],
                    s1=vi2[:, jc : jc + 1],
                    accum_out=qacc[:, jc, jc : jc + 1],
                )
                if jc == 0:
                    continue
                pt = ppsum.tile([1, BD], f32, tag="pt")
                for b in range(jc):
                    scrap = pscrap.tile([128, BD], f32r, tag="scrapr")
                    nc.vector._custom_dve(
                        op,
                        out=scrap[:, :],
                        in0=v1rep[:, j0 : j0 + BD],
                        in1=v2rep[:, j0 : j0 + BD],
                        s0=vi1[:, b : b + 1],
                        s1=vi2[:, b : b + 1],
                        accum_out=qacc[:, b, jc : jc + 1],
                    )
                    for h in range(BD // 512):
                        nc.tensor.matmul(
                            pt[:, h * 512 : (h + 1) * 512],
                            ones[:, :],
                            scrap[:, h * 512 : (h + 1) * 512],
                            start=(b == 0),
                            stop=(b == jc - 1),
                        )
                ct = pscrap.tile([1, BD], f32, tag="colbuf")
                nc.scalar.copy(ct[:, :], pt[:, :])
                nc.sync.dma_start(out=qcold.ap()[jc - 1, :], in_=ct[:, :])

            nc.sync.dma_start(out=qrowd.ap(), in_=qacc[:, :, :])

    nc.compile()
    return nc


def _abs_weighted_sums(q, x):
    """out_i = sum_j q_j * |x_i - x_j|, exact via sorting (float64)."""
    o = np.argsort(x, kind="stable")
    xs, qs = x[o], q[o]
    cq = np.cumsum(qs)
    cqx = np.cumsum(qs * xs)
    vals = xs * (2.0 * cq - cq[-1]) + cqx[-1] - 2.0 * cqx
    out = np.empty_like(vals)
    out[o] = vals
    return out


def _make_in_map(v1, v2, w, mode, c):
    rows = v1[c::8] if mode == "sym" else v1[c * ROWS : (c + 1) * ROWS]
    rows2 = v2[c::8] if mode == "sym" else v2[c * ROWS : (c + 1) * ROWS]
    vr1 = np.ascontiguousarray(rows).reshape(NIB, 128).T
    vr2 = np.ascontiguousarray(rows2).reshape(NIB, 128).T
    m = {
        "v1": v1,
        "v2": v2,
        "vipack": np.ascontiguousarray(
            np.concatenate([vr1, -vr1, vr2, -vr2], axis=1)
        ),
    }
    if mode == "weighted":
        m["w"] = w
    return m


def _run_device_qab(v1, v2, w, ones):
    from concourse.bass_utils import run_bass_kernel_spmd

    global LAST_RESULT
    mode = os.environ.get("DISCO_MODE") or ("sym" if ones else "weighted")
    nc = _build(mode)
    trace = os.environ.get("DISCO_TRACE", "0") == "1"
    in_maps = [_make_in_map(v1, v2, w, mode, c) for c in range(CORES)]
    res = run_bass_kernel_spmd(
        nc, in_maps, core_ids=list(range(CORES)), trace=trace
    )
    LAST_RESULT = res
    if mode == "sym":
        qab = np.empty(N, dtype=np.float64)
        colsum = np.zeros((NIB - 1, N // NIB), dtype=np.float64)
        for c, r in enumerate(res.results):
            qab[c::8] = r["qrow"].astype(np.float64).sum(axis=2).T.reshape(ROWS)
            colsum += r["qcol"].astype(np.float64)
        for band in range(1, NIB):
            qab[band * 1024 : (band + 1) * 1024] += colsum[band - 1]
        return qab
    parts = []
    for r in res.results:
        q = r["qab"].astype(np.float64)
        if q.ndim == 3:  # full fast path: [128, NIB, NJC] chunk partials
            q = q.sum(axis=2)
        parts.append(q.T.reshape(ROWS))  # [p, ib] -> row ib*128+p
    return np.concatenate(parts)


def kernel(var_1, var_2, normedweight, power):
    v1 = np.ascontiguousarray(np.asarray(var_1, dtype=np.float32))
    v2 = np.ascontiguousarray(np.asarray(var_2, dtype=np.float32))
    w = np.ascontiguousarray(np.asarray(normedweight, dtype=np.float32))
    p = int(np.asarray(power))
    ones = bool(np.all(w == np.float32(1.0)))

    qab = _run_device_qab(v1, v2, w, ones)

    v1d, v2d, wd = v1.astype(np.float64), v2.astype(np.float64), w.astype(np.float64)
    u = _abs_weighted_sums(wd, v1d) / N
    v = _abs_weighted_sums(wd, v2d) / N
    W = wd.sum()
    ga = (wd * u).mean()
    gb = (wd * v).mean()
    al = u - ga
    be = v - gb
    Qaa = W * v1d**2 - 2.0 * v1d * (wd * v1d).sum() + (wd * v1d**2).sum()
    Qbb = W * v2d**2 - 2.0 * v2d * (wd * v2d).sum() + (wd * v2d**2).sum()
    Duu = (wd * u * u).sum()
    Duv = (wd * u * v).sum()
    Dvv = (wd * v * v).sum()
    Rawu = _abs_weighted_sums(wd * u, v1d)
    Rawv = _abs_weighted_sums(wd * v, v1d)
    Rbwu = _abs_weighted_sums(wd * u, v2d)
    Rbwv = _abs_weighted_sums(wd * v, v2d)

    k = 2.0 * N - W
    SAA = Qaa - 2.0 * Rawu + Duu - al**2 * k
    SBB = Qbb - 2.0 * Rbwv + Dvv - be**2 * k
    SAB = qab - Rawv - Rbwu + Duv - al * be * k

    num = (np.abs(SAB) / N * wd).mean()
    denA = (SAA / N * wd).mean()
    denB = (SBB / N * wd).mean()
    EPS = 1e-12
    with np.errstate(all="ignore"):
        if p == 1:
            d = np.abs(denA * denB)
            out = num / np.sqrt(d + EPS)
        elif p == 2:
            d = np.abs(denA * denB)
            out = num**2 / (d + EPS)
        else:
            out = (num / np.sqrt(denA * denB) + EPS) ** p
    if np.isnan(out):
        out = 0.0
    out = max(out, 0.0)
    return np.float32(out)


# revision 29
# speedup vs baseline: 4278.2061x; 4278.2061x over previous
"""Distance-correlation (DisCo) loss kernel for Trainium2, sharded over 8 NeuronCores.

Math: reference computes NxN pairwise |vi-vj| matrices (a, b), weighted row
means, double-centering, then scalar reductions.  Everything except the
genuinely 2-D term

    Q_ab[i] = sum_j w_j * |v1_i - v1_j| * |v2_i - v2_j|

has an exact O(N log N) closed form on the host (sorted prefix sums for
weighted |.| row sums, polynomial identities for squared terms).  The device
computes Q_ab only, with rows i sharded across the 8 cores (1024 rows/core).

Device layout (per core): i on partitions (8 blocks of 128), j on the free
dim (2 chunks of 4096).  Tiles a=|v1_i-v1_j| and b=|v2_i-v2_j| are built by
the Vector engine (tensor_scalar: abs_max(in0-s, 0), fp32 2x mode) and the
Scalar engine (activation Abs with per-partition bias) splitting the j
columns, then a fused tensor_tensor_reduce (mult + free-dim add) produces
the row sums, chained across j chunks.  The weighted fallback multiplies b
by a broadcast w tile first.
"""

import functools
import os

import numpy as np

N = 8192
CORES = 8
ROWS = N // CORES          # 1024 rows per core
NIB = ROWS // 128          # 8 partition blocks per core
BCH = 1024                 # broadcast DMA chunk

LAST_RESULT = None         # BassKernelResults of the most recent launch


@functools.lru_cache(maxsize=1)
def _disco_op():
    """Fused DVE op: out = |in0-s0| * |in1-s1|, accum_out = sum(out).

    Registered at runtime into concourse.dve_ops; the uop table ships in
    the NEFF, so no firmware support is needed.  Exactly fills the 8-stage
    v3 DVE pipeline (2 subs, 2 negates, 2 maxes, 1 mul, 1 accum-add).
    """
    from operator import add

    import concourse.dve_ops as D
    from concourse.dve_spec import Spec, Src0, Src1, C0, C1, Zero, maxx, lower
    from concourse.dve_uop import DveOpSpec

    d1 = Src0 - C0
    d2 = Src1 - C1
    body = maxx(d1, Zero - d1) * maxx(d2, Zero - d2)

    def ref(in0, in1, s0, s1, imm2):
        b = (
            np.abs(in0.astype(np.float32) - s0) * np.abs(in1.astype(np.float32) - s1)
        ).astype(np.float32)
        return b, b.reshape(b.shape[0], -1).sum(axis=-1, keepdims=True)

    spec = Spec(body=body, accum=add, accum_init=Zero, reference=ref)
    name = "DISCO_ABSPROD_REDUCE"
    row = max(D._SUB_OPCODE_FOR_NAME.values()) + 1
    D._SUB_OPCODE_FOR_NAME[name] = row
    sha3 = DveOpSpec(
        name=name, opcode=row, uops=lower(spec, ver="v3"), rd1_en=True
    ).sha("v3")
    op = D.DveOp(name, spec, subdim=False, uops_sha={"v3": sha3})
    D.OPS.append(op)
    D.CUSTOM_DVE_SPECS[name] = spec
    return op


@functools.lru_cache(maxsize=3)
def _build(mode: str):
    """mode: 'sym' (w==1, symmetric block-triangle), 'full' (w==1, full
    matrix), or 'weighted' (general w)."""
    if mode == "sym":
        return _build_sym()
    import concourse.bacc as bacc
    import concourse.bass as bass
    import concourse.tile as tile
    from concourse import mybir

    weighted = mode == "weighted"
    f32 = mybir.dt.float32
    nc = bacc.Bacc("TRN2", target_bir_lowering=False, debug=False)

    # j-chunk size and the VectorE share of build columns, chosen to balance
    # VectorE vs ScalarE busy time per chunk while fitting SBUF.
    JC = 2048
    JD = 0
    NJC = N // JC

    v1d = nc.dram_tensor("v1", [N], f32, kind="ExternalInput")
    v2d = nc.dram_tensor("v2", [N], f32, kind="ExternalInput")
    wd = nc.dram_tensor("w", [N], f32, kind="ExternalInput") if weighted else None
    # vipack columns: [vi1 | -vi1 | vi2 | -vi2], each NIB wide, partition-major.
    vipackd = nc.dram_tensor("vipack", [128, 4 * NIB], f32, kind="ExternalInput")
    if weighted:
        qabd = nc.dram_tensor("qab", [128, NIB], f32, kind="ExternalOutput")
    else:
        qabd = nc.dram_tensor("qab", [128, NIB, NJC], f32, kind="ExternalOutput")

    def bcast(ap1d):
        return bass.AP(
            tensor=ap1d.tensor, offset=ap1d.offset, ap=[[0, 128]] + list(ap1d.ap)
        )

    i32 = mybir.dt.int32
    sub = mybir.AluOpType.subtract
    band = mybir.AluOpType.bitwise_and
    mult = mybir.AluOpType.mult
    add = mybir.AluOpType.add

    with tile.TileContext(nc) as tc:
        with (
            tc.tile_pool(name="singles", bufs=1) as singles,
            tc.tile_pool(name="ab", bufs=2) as pab,
            tc.tile_pool(name="scrap", bufs=1) as pscrap,
        ):
            v1rep = singles.tile([128, N], f32)
            v2rep = singles.tile([128, N], f32)
            reps = [(v1rep, v1d), (v2rep, v2d)]
            wrep = None
            if weighted:
                wrep = singles.tile([128, N], f32)
                reps.append((wrep, wd))
            for c in range(N // BCH):
                for rep, src in reps:
                    sap = src.ap()
                    nc.sync.dma_start(
                        out=rep[:, c * BCH : (c + 1) * BCH],
                        in_=bcast(sap[c * BCH : (c + 1) * BCH]),
                    )

            vipack = singles.tile([128, 4 * NIB], f32)
            nc.sync.dma_start(out=vipack[:, :], in_=vipackd.ap())
            vi1 = vipack[:, 0 * NIB : 1 * NIB]
            nvi1 = vipack[:, 1 * NIB : 2 * NIB]
            vi2 = vipack[:, 2 * NIB : 3 * NIB]
            nvi2 = vipack[:, 3 * NIB : 4 * NIB]

            if not weighted:
                # fused path: one custom DVE op per (i-block, chunk) computes
                # |v1_j - v1_i| * |v2_j - v2_i| and its row sum directly from
                # the replicated source rows -- no build tiles at all.
                op = _disco_op()
                qacc2 = singles.tile([128, NIB, NJC], f32)
                for jc in range(NJC):
                    for ib in range(NIB):
                        j0 = jc * JC
                        scrap = pscrap.tile([128, JC], f32)
                        nc.vector._custom_dve(
                            op,
                            out=scrap[:, :],
                            in0=v1rep[:, j0 : j0 + JC],
                            in1=v2rep[:, j0 : j0 + JC],
                            s0=vi1[:, ib : ib + 1],
                            s1=vi2[:, ib : ib + 1],
                            accum_out=qacc2[:, ib, jc : jc + 1],
                        )
                nc.sync.dma_start(out=qabd.ap(), in_=qacc2[:, :, :])
            else:
                qacc = singles.tile([128, NIB], f32)
                mask = None
                if JD > 0:
                    # 0x7FFFFFFF sign-clear mask: |x| on VectorE is a fp32
                    # subtract followed by an int32 bitwise_and against this.
                    mask = singles.tile([128, JD], i32)
                    nc.vector.memset(mask, 0x7FFFFFFF)

                for ib in range(NIB):
                    for jc in range(NJC):
                        j0 = jc * JC
                        ab = pab.tile([128, 2, JC], f32, tag="ab")
                        a = ab[:, 0, :]
                        b = ab[:, 1, :]
                        for t, (rep, vis, nvis) in enumerate(
                            ((v1rep, vi1, nvi1), (v2rep, vi2, nvi2))
                        ):
                            if JD > 0:
                                nc.vector.tensor_scalar(
                                    ab[:, t, :JD],
                                    rep[:, j0 : j0 + JD],
                                    vis[:, ib : ib + 1],
                                    None,
                                    sub,
                                )
                            nc.scalar.activation(
                                out=ab[:, t, JD:],
                                in_=rep[:, j0 + JD : j0 + JC],
                                func=mybir.ActivationFunctionType.Abs,
                                bias=nvis[:, ib : ib + 1],
                                scale=1.0,
                            )
                        if JD > 0:
                            for t in range(2):
                                nc.vector.tensor_tensor(
                                    ab[:, t, :JD].bitcast(i32),
                                    ab[:, t, :JD].bitcast(i32),
                                    mask[:, :],
                                    band,
                                )
                        wb = pab.tile([128, JC], f32, tag="wb")
                        nc.vector.tensor_tensor(wb, b, wrep[:, j0 : j0 + JC], mult)
                        scrap = pscrap.tile([128, JC], f32)
                        nc.vector.tensor_tensor(scrap, a, wb, mult)
                        # in-place copy whose op1 performs the free-dim
                        # reduction, chained across j chunks via scalar2 init
                        nc.vector.tensor_scalar(
                            scrap,
                            scrap,
                            1.0,
                            (0.0 if jc == 0 else qacc[:, ib : ib + 1]),
                            mult,
                            add,
                            accum_out=qacc[:, ib : ib + 1],
                        )

                nc.sync.dma_start(out=qabd.ap(), in_=qacc[:, :])

    nc.compile()
    return nc


def _build_sym(reps: int = 1):
    """Symmetric fast path (w == 1).

    M_ij = |v1_i-v1_j|*|v2_i-v2_j| is symmetric, so only the block upper
    triangle is computed.  Rows are interleaved across cores (core c owns
    global rows r with r % 8 == c), so each core's i-block b covers the
    global 1024-row band b.  Per band: the diagonal 1024-wide j-band is
    computed in full (row sums only); bands jc > b are computed once, with
    the custom op's accumulator providing the row sums and a ones-vector
    TensorE matmul (f32r, full rate) providing the partition-dim column
    sums into PSUM, accumulated over b and DMA'd out per jc.  0.5625x the
    elementwise work of the full matrix.
    """
    import concourse.bacc as bacc
    import concourse.bass as bass
    import concourse.tile as tile
    from concourse import mybir

    f32 = mybir.dt.float32
    f32r = mybir.dt.float32r
    BD = 1024  # band width
    NB = N // BD  # 8 bands == NIB
    assert NB == NIB

    nc = bacc.Bacc("TRN2", target_bir_lowering=False, debug=False)
    v1d = nc.dram_tensor("v1", [N], f32, kind="ExternalInput")
    v2d = nc.dram_tensor("v2", [N], f32, kind="ExternalInput")
    vipackd = nc.dram_tensor("vipack", [128, 4 * NIB], f32, kind="ExternalInput")
    qrowd = nc.dram_tensor("qrow", [128, NIB, NB], f32, kind="ExternalOutput")
    qcold = nc.dram_tensor("qcol", [NB - 1, BD], f32, kind="ExternalOutput")

    def bcast(ap1d):
        return bass.AP(
            tensor=ap1d.tensor, offset=ap1d.offset, ap=[[0, 128]] + list(ap1d.ap)
        )

    op = _disco_op()
    with tile.TileContext(nc) as tc:
        with (
            tc.tile_pool(name="singles", bufs=1) as singles,
            tc.tile_pool(name="scrap", bufs=4) as pscrap,
            tc.tile_pool(name="psum", bufs=2, space="PSUM") as ppsum,
        ):
            v1rep = singles.tile([128, N], f32)
            v2rep = singles.tile([128, N], f32)
            for c in range(N // BCH):
                for rep, src in ((v1rep, v1d), (v2rep, v2d)):
                    sap = src.ap()
                    nc.sync.dma_start(
                        out=rep[:, c * BCH : (c + 1) * BCH],
                        in_=bcast(sap[c * BCH : (c + 1) * BCH]),
                    )
            vipack = singles.tile([128, 4 * NIB], f32)
            nc.sync.dma_start(out=vipack[:, :], in_=vipackd.ap())
            vi1 = vipack[:, 0 * NIB : 1 * NIB]
            vi2 = vipack[:, 2 * NIB : 3 * NIB]

            qacc = singles.tile([128, NIB, NB], f32)
            nc.vector.memset(qacc, 0.0)
            ones_f = singles.tile([128, 1], f32)
            nc.vector.memset(ones_f, 1.0)
            ones = singles.tile([128, 1], f32r)
            nc.vector.tensor_copy(ones[:, :], ones_f[:, :])

            for _ in range(reps):
                for jc in range(NB):
                    j0 = jc * BD
                    # diagonal band: full row, row sums only
                    scrap = pscrap.tile([128, BD], f32, tag="scrap")
                    nc.vector._custom_dve(
                        op,
                        out=scrap[:, :],
                        in0=v1rep[:, j0 : j0 + BD],
                        in1=v2rep[:, j0 : j0 + BD],
                        s0=vi1[:, jc : jc + 1],
                        s1=vi2[:, jc : jc + 1],
                        accum_out=qacc[:, jc, jc : jc + 1],
                    )
                    if jc == 0:
                        continue
                    pt = ppsum.tile([1, BD], f32, tag="pt")
                    for b in range(jc):
                        scrap = pscrap.tile([128, BD], f32r, tag="scrapr")
                        nc.vector._custom_dve(
                            op,
                            out=scrap[:, :],
                            in0=v1rep[:, j0 : j0 + BD],
                            in1=v2rep[:, j0 : j0 + BD],
                            s0=vi1[:, b : b + 1],
                            s1=vi2[:, b : b + 1],
                            accum_out=qacc[:, b, jc : jc + 1],
                        )
                        for h in range(BD // 512):
                            nc.tensor.matmul(
                                pt[:, h * 512 : (h + 1) * 512],
                                ones[:, :],
                                scrap[:, h * 512 : (h + 1) * 512],
                                start=(b == 0),
                                stop=(b == jc - 1),
                            )
                    ct = pscrap.tile([1, BD], f32, tag="colbuf")
                    nc.scalar.copy(ct[:, :], pt[:, :])
                    nc.sync.dma_start(out=qcold.ap()[jc - 1, :], in_=ct[:, :])

            nc.sync.dma_start(out=qrowd.ap(), in_=qacc[:, :, :])

    nc.compile()
    return nc


def _abs_weighted_sums(q, x):
    """out_i = sum_j q_j * |x_i - x_j|, exact via sorting (float64)."""
    o = np.argsort(x, kind="stable")
    xs, qs = x[o], q[o]
    cq = np.cumsum(qs)
    cqx = np.cumsum(qs * xs)
    vals = xs * (2.0 * cq - cq[-1]) + cqx[-1] - 2.0 * cqx
    out = np.empty_like(vals)
    out[o] = vals
    return out


class _CachedRunner:
    """One-time-jitted SPMD executor (same lowering as bass2jax
    run_bass_via_pjrt, but the jitted callable is retained so repeat
    kernel() calls skip retracing/recompilation)."""

    def __init__(self, nc, n_cores=CORES):
        import jax
        from jax.experimental.shard_map import shard_map
        from jax.sharding import Mesh, PartitionSpec

        import concourse.mybir as mybir
        from concourse.bass2jax import (
            _bass_exec_p,
            install_neuronx_cc_hook,
            partition_id_tensor,
        )

        install_neuronx_cc_hook()
        self.n_cores = n_cores
        part_name = nc.partition_id_tensor.name if nc.partition_id_tensor else None
        in_names, out_names, out_avals, zero_outs = [], [], [], []
        for alloc in nc.m.functions[0].allocations:
            if not isinstance(alloc, mybir.MemoryLocationSet):
                continue
            name = alloc.memorylocations[0].name
            if alloc.kind == "ExternalInput":
                if name != part_name:
                    in_names.append(name)
            elif alloc.kind == "ExternalOutput":
                out_names.append(name)
                shape = tuple(alloc.tensor_shape)
                dtype = mybir.dt.np(alloc.dtype)
                out_avals.append(jax.core.ShapedArray(shape, dtype))
                zero_outs.append(np.zeros(shape, dtype))
        self.in_names, self.out_names = in_names, out_names
        self.zero_outs = zero_outs
        n_params = len(in_names)
        all_names = in_names + out_names
        if part_name is not None:
            all_names = all_names + [part_name]

        def _body(*args):
            operands = list(args)
            if part_name is not None:
                operands.append(partition_id_tensor())
            return tuple(
                _bass_exec_p.bind(
                    *operands,
                    out_avals=tuple(out_avals),
                    in_names=tuple(all_names),
                    out_names=tuple(out_names),
                    lowering_input_output_aliases=(),
                    sim_require_finite=True,
                    sim_require_nnan=True,
                    nc=nc,
                )
            )

        devices = jax.devices()[:n_cores]
        mesh = Mesh(np.asarray(devices), ("core",))
        nin = n_params + len(out_names)
        self.fn = jax.jit(
            shard_map(
                _body,
                mesh=mesh,
                in_specs=(PartitionSpec("core"),) * nin,
                out_specs=(PartitionSpec("core"),) * len(out_names),
                check_rep=False,
            ),
            donate_argnums=tuple(range(n_params, nin)),
            keep_unused=True,
        )

    def run(self, in_maps):
        n = self.n_cores
        concat_in = [
            np.concatenate([np.asarray(in_maps[c][k]) for c in range(n)], axis=0)
            for k in self.in_names
        ]
        concat_zero = [np.concatenate([z] * n, axis=0) for z in self.zero_outs]
        outs = [np.asarray(o) for o in self.fn(*concat_in, *concat_zero)]
        per_core = []
        for c in range(n):
            d = {}
            for k, o in zip(self.out_names, outs):
                m = o.shape[0] // n
                d[k] = o[c * m : (c + 1) * m]
            per_core.append(d)
        return per_core


_RUNNER_CACHE = {}


def _make_in_map(v1, v2, w, mode, c):
    rows = v1[c::8] if mode == "sym" else v1[c * ROWS : (c + 1) * ROWS]
    rows2 = v2[c::8] if mode == "sym" else v2[c * ROWS : (c + 1) * ROWS]
    vr1 = np.ascontiguousarray(rows).reshape(NIB, 128).T
    vr2 = np.ascontiguousarray(rows2).reshape(NIB, 128).T
    m = {
        "v1": v1,
        "v2": v2,
        "vipack": np.ascontiguousarray(
            np.concatenate([vr1, -vr1, vr2, -vr2], axis=1)
        ),
    }
    if mode == "weighted":
        m["w"] = w
    return m


def _run_device_qab(v1, v2, w, ones):
    global LAST_RESULT
    mode = os.environ.get("DISCO_MODE") or ("sym" if ones else "weighted")
    nc = _build(mode)
    trace = os.environ.get("DISCO_TRACE", "0") == "1"
    in_maps = [_make_in_map(v1, v2, w, mode, c) for c in range(CORES)]
    if trace or os.environ.get("DISCO_NO_RUNNER_CACHE", "0") == "1":
        from concourse.bass_utils import run_bass_kernel_spmd

        res = run_bass_kernel_spmd(
            nc, in_maps, core_ids=list(range(CORES)), trace=trace
        )
        LAST_RESULT = res
        results = res.results
    else:
        runner = _RUNNER_CACHE.get(mode)
        if runner is None:
            runner = _CachedRunner(nc)
            _RUNNER_CACHE[mode] = runner
        results = runner.run(in_maps)

    if mode == "sym":
        qab = np.empty(N, dtype=np.float64)
        colsum = np.zeros((NIB - 1, N // NIB), dtype=np.float64)
        for c, r in enumerate(results):
            qab[c::8] = r["qrow"].astype(np.float64).sum(axis=2).T.reshape(ROWS)
            colsum += r["qcol"].astype(np.float64)
        for band in range(1, NIB):
            qab[band * 1024 : (band + 1) * 1024] += colsum[band - 1]
        return qab
    parts = []
    for r in results:
        q = r["qab"].astype(np.float64)
        if q.ndim == 3:  # full fast path: [128, NIB, NJC] chunk partials
            q = q.sum(axis=2)
        parts.append(q.T.reshape(ROWS))  # [p, ib] -> row ib*128+p
    return np.concatenate(parts)


def kernel(var_1, var_2, normedweight, power):
    v1 = np.ascontiguousarray(np.asarray(var_1, dtype=np.float32))
    v2 = np.ascontiguousarray(np.asarray(var_2, dtype=np.float32))
    w = np.ascontiguousarray(np.asarray(normedweight, dtype=np.float32))
    p = int(np.asarray(power))
    ones = bool(np.all(w == np.float32(1.0)))

    qab = _run_device_qab(v1, v2, w, ones)

    v1d, v2d, wd = v1.astype(np.float64), v2.astype(np.float64), w.astype(np.float64)
    u = _abs_weighted_sums(wd, v1d) / N
    v = _abs_weighted_sums(wd, v2d) / N
    W = wd.sum()
    ga = (wd * u).mean()
    gb = (wd * v).mean()
    al = u - ga
    be = v - gb
    Qaa = W * v1d**2 - 2.0 * v1d * (wd * v1d).sum() + (wd * v1d**2).sum()
    Qbb = W * v2d**2 - 2.0 * v2d * (wd * v2d).sum() + (wd * v2d**2).sum()
    Duu = (wd * u * u).sum()
    Duv = (wd * u * v).sum()
    Dvv = (wd * v * v).sum()
    Rawu = _abs_weighted_sums(wd * u, v1d)
    Rawv = _abs_weighted_sums(wd * v, v1d)
    Rbwu = _abs_weighted_sums(wd * u, v2d)
    Rbwv = _abs_weighted_sums(wd * v, v2d)

    k = 2.0 * N - W
    SAA = Qaa - 2.0 * Rawu + Duu - al**2 * k
    SBB = Qbb - 2.0 * Rbwv + Dvv - be**2 * k
    SAB = qab - Rawv - Rbwu + Duv - al * be * k

    num = (np.abs(SAB) / N * wd).mean()
    denA = (SAA / N * wd).mean()
    denB = (SBB / N * wd).mean()
    EPS = 1e-12
    with np.errstate(all="ignore"):
        if p == 1:
            d = np.abs(denA * denB)
            out = num / np.sqrt(d + EPS)
        elif p == 2:
            d = np.abs(denA * denB)
            out = num**2 / (d + EPS)
        else:
            out = (num / np.sqrt(denA * denB) + EPS) ** p
    if np.isnan(out):
        out = 0.0
    out = max(out, 0.0)
    return np.float32(out)
